# revision 37
# baseline (speedup 1.0000x reference)
"""BiModal attention kernel for Trainium2 (8 NeuronCores, data-parallel over batch).

Per core (one batch b): x, y: [2048, 128] fp32.
  S = x @ y.T                    (f32r matmuls, [2048, 2048])
  E = exp(S)                     (unshifted; |S| <~ 67 so exp stays in range)
  a1 = (E @ y) / rowsum(E) * x
  a2 = (E.T @ x) / colsum(E) * y
  out = concat([a1, a2], -1)     ([2048, 256])

Rows are relabeled r = 16*p + b (p = SBUF partition, b = block index) so DRAM
transfers are contiguous per partition; applied consistently to s and t.
Score columns are enumerated c = tb*128 + tp (t-row = 16*tp + tb), matching
the yT streaming order.

v2 (row-major) structure:
  - xT/yT built by PE transposes from the f32 loads (exact, no hi/lo DMA).
  - one iteration per (row i, panel ct): S (2 f32r matmuls) -> exp -> o2
    chunk matmuls for the previous half-row -> lagged o1 chunk matmuls.
  - after both panels of row i: ONE xbar transpose E[:,i,:] -> ET[:,i,:,:]
    ([128,2048], 4KB contiguous on both sides, dst groups uniform 256B).
  - col-sum partials (l2p) per row on DVE in bf16 (2x mode), f32 fold.
  - o1 accumulates in 2 PSUM banks (q = s-quarters, two banks ping-pong,
    drained to bf16 o1t as each quarter completes); o2 in 4 banks.
  - o1/o2 return to [s-part, d] via full-width xbar transposes; gating on
    DVE; 4 half-MB stores on the ACT HWDGE ring.
"""
import sys

sys.path.insert(0, "/opt/trn_rl_repo")

import os
import numpy as np

import concourse.bass as bass
import concourse.mybir as mybir
import concourse.tile as tile
from concourse.tile_rust import add_dep_helper
from concourse import bacc
from concourse.bass_utils import run_bass_kernel_spmd
from concourse.masks import make_identity

f32 = mybir.dt.float32
f32r = mybir.dt.float32r
bf16 = mybir.dt.bfloat16

B = 8
S = 2048
D = 128
P = 128
NB = S // P          # 16 row blocks
NP = 2               # panels
PW = S // NP         # panel width (1024)
PB = PW // P         # t-blocks per panel (8)

_NC_CACHE = None
LAST_EXEC_NS = None

DEBUG = bool(int(os.environ.get("KERNEL_DEBUG", "0")))


def _build_program(nc):
    x_d = nc.dram_tensor("x", [S, D], f32, kind="ExternalInput").ap()
    y_d = nc.dram_tensor("y", [S, D], f32, kind="ExternalInput").ap()
    out_d = nc.dram_tensor("out", [S, 2 * D], f32, kind="ExternalOutput").ap()

    x_dv = x_d.rearrange("(p b) d -> p b d", p=P)      # [128, 16, 128]
    y_dv = y_d.rearrange("(p b) d -> p b d", p=P)
    out_dv = out_d.rearrange("(p b) c -> p b c", p=P)  # [128, 16, 256]

    Exp = mybir.ActivationFunctionType.Exp
    MUL = mybir.AluOpType.mult
    ADD = mybir.AluOpType.add
    AX = mybir.AxisListType.X

    with tile.TileContext(nc) as tc:
        with (
            tc.tile_pool(name="sb", bufs=1) as sb,
            tc.tile_pool(name="shp", bufs=1) as shp,
            tc.tile_pool(name="shq", bufs=1) as shq,
            tc.tile_pool(name="ps", bufs=1, space="PSUM") as ps,
        ):
            # ---- persistent SBUF ----
            x_sb = shp.tile([P, NB, D], f32, tag="P", name="x_f32")
            y_sb = shq.tile([P, NB, D], f32, tag="Q", name="y_f32")
            xT = sb.tile([P, NB, P], f32r, tag="xT")       # [d, i, sp]
            yT = sb.tile([P, NB, P], f32r, tag="yT")       # [d, tb, tp]
            x_bf = sb.tile([P, NB, D], bf16, tag="x_bf")
            y_bf = sb.tile([P, NB, D], bf16, tag="y_bf")
            E0 = sb.tile([P, NB, PW], bf16, tag="E0")      # [sp, i, c<1024]
            E1 = sb.tile([P, NB, PW], bf16, tag="E1")      # [sp, i, c>=1024]
            E_ = [E0, E1]
            ET = sb.tile([P, NB, NB, P], bf16, tag="ET")   # [tp, i, tb, sp]
            o1t = sb.tile([P, S], bf16, tag="o1t")         # [d, s]
            o2t = sb.tile([P, S], bf16, tag="o2t")         # [d, c]
            o1s = sb.tile([P, NB, D], bf16, tag="o1s")     # [sp, i, d]
            o2s = sb.tile([P, NB, D], bf16, tag="o2s")     # [tp, tb, d]
            ident = sb.tile([P, P], f32, tag="ident")
            warm = sb.tile([P, 2, P], bf16, tag="warm")
            yr = sb.tile([P, NB, D], bf16, tag="yr")       # y_bf * r2
            l1p = sb.tile([P, 2 * NB], f32, tag="l1p")     # [sp, git]
            l2p = sb.tile([P, NB, NB], bf16, tag="l2p")    # [tp, i, tb]
            l1 = sb.tile([P, NB], f32, tag="l1")
            l2 = sb.tile([P, NB], f32, tag="l2")
            r1 = sb.tile([P, NB], f32, tag="r1")
            r2 = sb.tile([P, NB], f32, tag="r2")

            # ---- PSUM (8 banks exactly) ----
            s_psA = ps.tile([P, 2, 512], f32, tag="A0")    # ct=0 pair
            s_psB = ps.tile([P, 2, 512], f32, tag="A1")    # ct=1 pair
            s_pair = [s_psA, s_psB]
            o1_ps = ps.tile([P, 512], f32, tag="B")        # 1 bank (qs sequential)
            o2_ps = ps.tile([P, 3, 512], f32, tag="C")     # 3 banks (chunks 0-2)
            pTs = [ps.tile([P, 4, P], f32, tag="B", name="pT0"),
                   ps.tile([P, 4, P], f32, tag="C", name="pT1")]

            last_pe = [None]

            def pe_chain(mm):
                if last_pe[0] is not None:
                    add_dep_helper(mm.ins, last_pe[0].ins, sync=False,
                                   reason="keep PE emission order")
                last_pe[0] = mm
                return mm

            # ---- warmup: unthrottle HAM during the loads ----
            make_identity(nc, ident[:])
            nc.gpsimd.memset(warm[:], 0)
            wf = warm[:].rearrange("p a b -> p (a b)")     # [128, 256]
            for w in range(20):
                pe_chain(nc.tensor.matmul(s_psA[:, 0, 0:256], warm[:, 0, :],
                                          wf, start=True, stop=True))

            def filler(ct, n=1):
                # tiny dummy matmuls into the S banks the next real S pair
                # will overwrite: keeps the PE activity monitor at full clock
                # without extra PSUM. WAR/WAW deps mirror the real S pair's.
                for _ in range(n):
                    pe_chain(nc.tensor.matmul(
                        s_pair[ct][:, 0, 0:P], warm[:, 0, :], wf[:, 0:P],
                        start=True, stop=True))

            # ---- loads (both HWDGE rings in parallel) ----
            nc.sync.dma_start(y_sb[:, 0:PB], y_dv[:, 0:PB])
            nc.scalar.dma_start(x_sb[:, 0:PB], x_dv[:, 0:PB])
            nc.sync.dma_start(y_sb[:, PB:NB], y_dv[:, PB:NB])
            nc.scalar.dma_start(x_sb[:, PB:NB], x_dv[:, PB:NB])

            # ---- prologue: exact xT/yT via PE transposes ----
            pp = [0]

            def prologue_T(v_sb, vT, v_bf, half):
                for k in range(2):
                    pT = pTs[pp[0] % 2]
                    pp[0] += 1
                    b0 = half * PB + k * 4
                    for b4 in range(4):
                        pe_chain(nc.tensor.transpose(
                            pT[:, b4, :], v_sb[:, b0 + b4, :], ident[:]))
                    # transpose-mode doesn't count as PE-busy for the clock
                    # gate; feed it a real matmul per batch
                    pe_chain(nc.tensor.matmul(
                        s_psA[:, 0, 0:256], warm[:, 0, :], wf,
                        start=True, stop=True))
                    nc.scalar.copy(vT[:, b0:b0 + 4, :], pT[:])
                sl = slice(half * PB, (half + 1) * PB)
                nc.vector.tensor_copy(v_bf[:, sl], v_sb[:, sl])

            prologue_T(y_sb, yT, y_bf, 0)
            prologue_T(x_sb, xT, x_bf, 0)
            prologue_T(y_sb, yT, y_bf, 1)
            prologue_T(x_sb, xT, x_bf, 1)

            # ---- o1 chunk schedule ----
            # chunk (tb, q) = one N=512 matmul over ET[:, 4q:4q+4, tb, :];
            # needs T(4q+3), emitted at git 2*(4q+3)+1 -> ready 8q+9.
            NG = 2 * NB
            pops = [[] for _ in range(NG + 1)]
            queue = []
            for q in range(4):
                for tb in range(NB):
                    queue.append((8 * q + 10, tb, q))
            queue.sort(key=lambda t: t[0])
            qi = 0
            for g in range(NG + 1):
                cap = 3 if g < NG else 64
                while qi < len(queue) and (queue[qi][0] <= g or g == NG) \
                        and cap > 0:
                    pops[g].append(queue[qi][1:])
                    qi += 1
                    cap -= 1
            assert qi == len(queue)

            o1_count = [0] * 4

            def emit_o1(tb, q):
                c = o1_count[q]
                o1_count[q] += 1
                pe_chain(nc.tensor.matmul(
                    o1_ps[:], y_bf[:, tb, :],
                    ET[:, 4 * q:4 * q + 4, tb, :],
                    start=(c == 0), stop=(c == NB - 1)))
                if o1_count[q] == NB and q < 3:
                    nc.vector.tensor_copy(o1t[:, q * 512:(q + 1) * 512],
                                          o1_ps[:])
                    if q == 1:
                        nc.sync.dma_start_transpose(o1s[:, 0:PB, :],
                                                    o1t[:, 0:PW])

            def emit_o2_for(prev_git):
                # chunk 3 is deferred to the tail (its PSUM bank hosts the
                # S ping-pong); chunks 0-2 accumulate in-loop.
                pi, pct = divmod(prev_git, 2)
                for qq in (2 * pct, 2 * pct + 1):
                    if qq == 3:
                        continue
                    cw = qq * 512 - pct * PW
                    pe_chain(nc.tensor.matmul(
                        o2_ps[:, qq, :], x_bf[:, pi, :],
                        E_[pct][:, pi, cw:cw + 512],
                        start=(pi == 0), stop=(pi == NB - 1)))

            outP = shp.tile([P, NB, D], f32, tag="P", name="outP")
            outQ = shq.tile([P, NB, D], f32, tag="Q", name="outQ")

            # ---- main: 16 rows x 2 panels ----
            yTf = yT[:].rearrange("p b d -> p (b d)")      # [128, 2048]
            for i in range(NB):
                for ct in range(NP):
                    git = 2 * i + ct
                    c0 = ct * PW
                    sps = s_pair[ct]
                    if len(pops[git]) == 0:
                        filler(ct, 3)
                    pe_chain(nc.tensor.matmul(
                        sps[:, 0, :], xT[:, i, :], yTf[:, c0:c0 + 512],
                        start=True, stop=True))
                    pe_chain(nc.tensor.matmul(
                        sps[:, 1, :], xT[:, i, :],
                        yTf[:, c0 + 512:c0 + 1024], start=True, stop=True))
                    nc.scalar.activation(
                        E_[ct][:, i, :],
                        sps[:].rearrange("p a b -> p (a b)"),
                        Exp, accum_out=l1p[:, git:git + 1])
                    nc.sync.dma_start_transpose(
                        ET[:, i, ct * PB:(ct + 1) * PB, :], E_[ct][:, i, :])
                    if git >= 1:
                        emit_o2_for(git - 1)
                    for (tb, q) in pops[git]:
                        emit_o1(tb, q)
                    with nc.allow_low_precision("l2 partials in bf16"):
                        if git >= 1:
                            pi, pct = divmod(git - 1, 2)
                            nc.vector.tensor_reduce(
                                l2p[:, pi, pct * PB:(pct + 1) * PB],
                                ET[:, pi, pct * PB:(pct + 1) * PB, :],
                                axis=AX, op=ADD)
                    if git == NG - 1:
                        # deferred o2 chunk 3 starts here: E1 is complete
                        # through row 14; row 15's contribution needs this
                        # git's exp and lands below in program order
                        for pi in range(NB - 1):
                            pe_chain(nc.tensor.matmul(
                                s_psA[:, 0, :], x_bf[:, pi, :],
                                E1[:, pi, 512:1024],
                                start=(pi == 0), stop=False))
                    if git == NG - 1:
                        with nc.allow_low_precision("l2 fold"):
                            pass
                        nc.vector.tensor_reduce(
                            l2[:, 0:PB],
                            l2p[:, :, 0:PB].rearrange("p a b -> p b a"),
                            axis=AX, op=ADD)
                        nc.vector.reciprocal(r2[:, 0:PB], l2[:, 0:PB])
                    if git == 20:
                        # rows 0-7 of l1 are complete; a1 half-1 can be
                        # normalized and gated while the loop continues
                        nc.vector.tensor_reduce(
                            l1[:, 0:PB],
                            l1p[:, 0:NB].rearrange("p (a b) -> p a b", a=PB),
                            axis=AX, op=ADD)
                        nc.vector.reciprocal(r1[:, 0:PB], l1[:, 0:PB])
                    if 26 <= git <= 29:
                        for b in (2 * (git - 26), 2 * (git - 26) + 1):
                            nc.vector.scalar_tensor_tensor(
                                outP[:, b, :], o1s[:, b, :], r1[:, b:b + 1],
                                x_bf[:, b, :], op0=MUL, op1=MUL)

            # ---- tail ----
            emit_o2_for(NG - 1)
            # a1 rows 0-7 were gated in-loop: ship them immediately
            nc.sync.dma_start(out_dv[:, 0:PB, 0:D], outP[:, 0:PB, :])

            # final contribution of deferred o2 chunk 3 (row 15)
            pe_chain(nc.tensor.matmul(
                s_psA[:, 0, :], x_bf[:, NB - 1, :], E1[:, NB - 1, 512:1024],
                start=False, stop=True))
            for (tb, q) in pops[NG]:
                emit_o1(tb, q)

            # a2 = o2s * (y_bf * r2): per-row scales on ACT, wide DVE
            # multiplies; half-1 of everything ships as early as possible
            nc.scalar.copy(o2t[:, 0:PW],
                           o2_ps[:, 0:2].rearrange("p a b -> p (a b)"))
            nc.sync.dma_start_transpose(o2s[:, 0:PB, :], o2t[:, 0:PW])
            for b in range(PB):
                nc.scalar.mul(yr[:, b, :], y_bf[:, b, :], r2[:, b:b + 1])
            nc.scalar.copy(o2t[:, PW:1536], o2_ps[:, 2, :])
            nc.scalar.copy(o2t[:, 1536:2048], s_psA[:, 0, :])
            nc.sync.dma_start_transpose(o2s[:, PB:NB, :], o2t[:, PW:S])

            with nc.allow_low_precision("l2 partials in bf16"):
                nc.vector.tensor_reduce(
                    l2p[:, NB - 1, PB:NB], ET[:, NB - 1, PB:NB, :],
                    axis=AX, op=ADD)
            nc.vector.tensor_reduce(
                l2[:, PB:NB],
                l2p[:, :, PB:NB].rearrange("p a b -> p b a"), axis=AX, op=ADD)
            nc.vector.reciprocal(r2[:, PB:NB], l2[:, PB:NB])
            nc.vector.tensor_reduce(
                l1[:, PB:NB],
                l1p[:, NB:2 * NB].rearrange("p (a b) -> p a b", a=PB),
                axis=AX, op=ADD)
            nc.vector.reciprocal(r1[:, PB:NB], l1[:, PB:NB])

            nc.vector.tensor_tensor(outQ[:, 0:PB, :], o2s[:, 0:PB, :],
                                    yr[:, 0:PB, :], op=MUL)
            nc.scalar.dma_start(out_dv[:, 0:PB, D:2 * D], outQ[:, 0:PB, :])
            # q3 drain deferred to here so it doesn't head-of-line block the
            # DVE queue ahead of the a2 half-1 chain
            nc.vector.tensor_copy(o1t[:, 1536:2048], o1_ps[:])
            nc.sync.dma_start_transpose(o1s[:, PB:NB, :], o1t[:, PW:S])
            for b in range(PB, NB):
                nc.scalar.mul(yr[:, b, :], y_bf[:, b, :], r2[:, b:b + 1])
            for b in range(PB, NB):
                nc.vector.scalar_tensor_tensor(
                    outP[:, b, :], o1s[:, b, :], r1[:, b:b + 1], x_bf[:, b, :],
                    op0=MUL, op1=MUL)
            nc.scalar.dma_start(out_dv[:, PB:NB, 0:D], outP[:, PB:NB, :])
            nc.vector.tensor_tensor(outQ[:, PB:NB, :], o2s[:, PB:NB, :],
                                    yr[:, PB:NB, :], op=MUL)
            nc.scalar.dma_start(out_dv[:, PB:NB, D:2 * D], outQ[:, PB:NB, :])

            if DEBUG:
                dbg_specs = [
                    ("dbg_xT", xT, [P, NB, P], f32, True),
                    ("dbg_yT", yT, [P, NB, P], f32, True),
                    ("dbg_E0", E0, [P, NB, PW], bf16, False),
                    ("dbg_E1", E1, [P, NB, PW], bf16, False),
                    ("dbg_ET", ET, [P, NB, NB, P], bf16, False),
                    ("dbg_o1t", o1t, [P, S], bf16, False),
                    ("dbg_o2t", o2t, [P, S], bf16, False),
                    ("dbg_o1s", o1s, [P, NB, D], bf16, False),
                    ("dbg_o2s", o2s, [P, NB, D], bf16, False),
                    ("dbg_l1p", l1p, [P, 2 * NB], f32, False),
                    ("dbg_l2p", l2p, [P, NB, NB], bf16, False),
                    ("dbg_l1", l1, [P, NB], f32, False),
                    ("dbg_l2", l2, [P, NB], f32, False),
                ]
                for name, t, shp_, dt_, cast_ in dbg_specs:
                    dd = nc.dram_tensor(name, shp_, dt_,
                                        kind="ExternalOutput").ap()
                    src = t[:].bitcast(f32) if cast_ else t[:]
                    nc.sync.dma_start(dd, src)

    nc.compile()
    return nc


def _get_nc():
    global _NC_CACHE
    if _NC_CACHE is None:
        nc = bacc.Bacc("TRN2", target_bir_lowering=False, debug=False,
                       num_devices=B)
        _NC_CACHE = _build_program(nc)
    return _NC_CACHE


def kernel(x, y):
    global LAST_EXEC_NS
    nc = _get_nc()
    x = np.asarray(x, dtype=np.float32)
    y = np.asarray(y, dtype=np.float32)
    in_maps = [
        {"x": np.ascontiguousarray(x[b]), "y": np.ascontiguousarray(y[b])}
        for b in range(B)
    ]
    trace = bool(int(os.environ.get("KERNEL_TRACE", "0")))
    res = run_bass_kernel_spmd(nc, in_maps, list(range(B)), trace=trace)
    LAST_EXEC_NS = res.exec_time_ns
    return np.stack([res.results[b]["out"] for b in range(B)], axis=0)
